# revision 1
# baseline (speedup 1.0000x reference)
"""Trainium2 Bass kernel for DiceLoss (hard-argmax dice, ignore background, mean).

Problem (hardcoded shapes):
  y_true: [16, 512, 512] int32 in [0, 8)
  y_pred: [16, 8, 512, 512] float32
  out   : scalar float32 = mean over classes 1..7 of
          (2*tp + eps) / (2*tp + fp + fn + eps)
  with pred_cls = argmax_c y_pred, one-hot tp/fp/fn sums over all pixels.

Strategy (8 NeuronCores, data-parallel over batch):
  - Each core processes 2 of the 16 batch images (SPMD, same NEFF).
  - Per core the image planes are streamed in [128, 1024] chunks
    (contiguous HBM -> optimal DMA).
  - VectorE (DVE): 7-op pairwise max tree -> m = max over channels, then per
    class c in 1..7 a fused scalar_tensor_tensor:
        pred_c = (y_pred[c] == m)  (bf16 mask) with accum_out = per-partition
        sum(pred_c) -> pred counts come for free.
  - GpSimd: per class gt_c = (y_true == c) tensor_scalar mask (+accum_out for
    gt counts). Runs concurrently with DVE (DVE stays in 1x tensor-tensor
    mode so the shared SBUF port pair is never contended).
  - ScalarE (ACT): int32 -> f32 convert of labels.
  - TensorE (PE): tp_c = sum(pred_c * gt_c) via the diagonal of
    pred_c[:, s*128:(s+1)*128]^T @ gt_c[:, s*128:(s+1)*128] accumulated in
    PSUM over all subtiles and chunks. Host reads the trace.
  - Host: sums the 8 cores' partial sums (exact small integers in f32) and
    forms the dice mean in float32, matching the reference arithmetic.
"""

import numpy as np

EPS = 1e-05

# Problem geometry (hardcoded per the harness contract).
N_CORES = 8
NB = 2          # batch images per core
C = 8           # classes
P = 128         # SBUF partitions
F = 1024        # free-dim elements per chunk
NCHUNK = 2      # chunks per image plane (512*512 = 2*128*1024)
CHUNKS = NB * NCHUNK
NSUB = F // 128  # 128-wide subtiles per chunk for the PE tp matmuls

_CACHED_NC = None


def build_bass():
    """Build the Bass kernel (same NEFF for all 8 cores)."""
    from contextlib import ExitStack

    import concourse.bacc as bacc
    import concourse.tile as tile
    from concourse import mybir

    nc = bacc.Bacc(None, target_bir_lowering=False)

    yp = nc.dram_tensor(
        "yp", [NB, C, NCHUNK, P, F], mybir.dt.float32, kind="ExternalInput"
    )
    yt = nc.dram_tensor("yt", [NB, NCHUNK, P, F], mybir.dt.int32, kind="ExternalInput")
    # tp partials: per class a [128, 128] PSUM accumulator; host takes trace().
    tp_out = nc.dram_tensor("tp_out", [7, P, 128], mybir.dt.float32, kind="ExternalOutput")
    # per-(chunk, class) per-partition pred / gt counts
    pa_out = nc.dram_tensor("pa_out", [P, CHUNKS * 7], mybir.dt.float32, kind="ExternalOutput")
    ga_out = nc.dram_tensor("ga_out", [P, CHUNKS * 7], mybir.dt.float32, kind="ExternalOutput")

    with tile.TileContext(nc) as tc, ExitStack() as ctx:
        chpool = ctx.enter_context(tc.tile_pool(name="ch", bufs=2))
        tpool = ctx.enter_context(tc.tile_pool(name="tt", bufs=2))
        mpool = ctx.enter_context(tc.tile_pool(name="mx", bufs=2))
        mtmp = ctx.enter_context(tc.tile_pool(name="mtmp", bufs=5))
        maskp = ctx.enter_context(tc.tile_pool(name="mask", bufs=3))
        gtpool = ctx.enter_context(tc.tile_pool(name="gt", bufs=3))
        accp = ctx.enter_context(tc.tile_pool(name="acc", bufs=1))
        psump = ctx.enter_context(tc.tile_pool(name="psum", bufs=1, space="PSUM"))

        pred_acc = accp.tile([P, CHUNKS * 7], mybir.dt.float32, name="pred_acc")
        gt_acc = accp.tile([P, CHUNKS * 7], mybir.dt.float32, name="gt_acc")
        psums = [
            psump.tile([P, 128], mybir.dt.float32, name=f"ps{c}", tag=f"ps{c}")
            for c in range(1, C)
        ]

        chunk_idx = 0
        for n in range(NB):
            for j in range(NCHUNK):
                ch = []
                for c in range(C):
                    tl = chpool.tile([P, F], mybir.dt.float32, name=f"ch{c}", tag=f"ch{c}")
                    nc.sync.dma_start(out=tl, in_=yp[n, c, j])
                    ch.append(tl)
                tt = tpool.tile([P, F], mybir.dt.int32, name="t", tag="t")
                nc.sync.dma_start(out=tt, in_=yt[n, j])
                # labels to bf16 (exact for 0..7) on the otherwise idle ScalarE;
                # 16-bit source lets the gt tensor_scalar below hit 4x perf mode
                tf = tpool.tile([P, F], mybir.dt.bfloat16, name="tf", tag="tf")
                nc.scalar.copy(out=tf, in_=tt)

                # ---- max tree (DVE, all 1x tensor-tensor ops) ----
                m01 = mtmp.tile([P, F], mybir.dt.float32, name="m01", tag="mt")
                nc.vector.tensor_max(m01, ch[0], ch[1])
                m23 = mtmp.tile([P, F], mybir.dt.float32, name="m23", tag="mt")
                nc.vector.tensor_max(m23, ch[2], ch[3])
                m45 = mtmp.tile([P, F], mybir.dt.float32, name="m45", tag="mt")
                nc.vector.tensor_max(m45, ch[4], ch[5])
                m67 = mtmp.tile([P, F], mybir.dt.float32, name="m67", tag="mt")
                nc.vector.tensor_max(m67, ch[6], ch[7])
                m0123 = mtmp.tile([P, F], mybir.dt.float32, name="m0123", tag="mt")
                nc.vector.tensor_max(m0123, m01, m23)
                m4567 = mtmp.tile([P, F], mybir.dt.float32, name="m4567", tag="mt")
                nc.vector.tensor_max(m4567, m45, m67)
                m = mpool.tile([P, F], mybir.dt.float32, name="m", tag="m")
                nc.vector.tensor_max(m, m0123, m4567)

                # ---- per-class masks + fused counts + PE tp ----
                for c in range(1, C):
                    col = chunk_idx * 7 + (c - 1)
                    pred = maskp.tile([P, F], mybir.dt.bfloat16, name=f"pred{c}", tag="pred")
                    nc.vector.scalar_tensor_tensor(
                        out=pred,
                        in0=ch[c],
                        scalar=0.0,
                        in1=m,
                        op0=mybir.AluOpType.add,
                        op1=mybir.AluOpType.is_equal,
                        accum_out=pred_acc[:, col : col + 1],
                    )
                    gt = gtpool.tile([P, F], mybir.dt.bfloat16, name=f"gt{c}", tag="gt")
                    # NOTE: measured on HW, nc.gpsimd.tensor_scalar is ~16us per
                    # [128,1024] op (software-dispatch bound) — DVE tensor_scalar
                    # on a bf16 source runs in 4x perf mode and is ~60x faster,
                    # with the gt count fused in via accum_out.
                    nc.vector.tensor_scalar(
                        out=gt,
                        in0=tf,
                        scalar1=float(c),
                        scalar2=0.0,
                        op0=mybir.AluOpType.is_equal,
                        op1=mybir.AluOpType.add,
                        accum_out=gt_acc[:, col : col + 1],
                    )
                    for s in range(NSUB):
                        nc.tensor.matmul(
                            psums[c - 1][:, :],
                            lhsT=pred[:, s * 128 : (s + 1) * 128],
                            rhs=gt[:, s * 128 : (s + 1) * 128],
                            start=(chunk_idx == 0 and s == 0),
                            stop=(chunk_idx == CHUNKS - 1 and s == NSUB - 1),
                        )
                chunk_idx += 1

        for c in range(7):
            tps = accp.tile([P, 128], mybir.dt.float32, name=f"tps{c}", tag=f"tps{c}")
            nc.scalar.copy(out=tps, in_=psums[c])
            nc.sync.dma_start(out=tp_out[c], in_=tps)
        nc.sync.dma_start(out=pa_out[:], in_=pred_acc)
        nc.sync.dma_start(out=ga_out[:], in_=gt_acc)

    nc.finalize()
    return nc


def _get_bass():
    global _CACHED_NC
    if _CACHED_NC is None:
        _CACHED_NC = build_bass()
    return _CACHED_NC


def make_in_maps(y_true, y_pred):
    yp = np.ascontiguousarray(np.asarray(y_pred, dtype=np.float32))
    yt = np.ascontiguousarray(np.asarray(y_true, dtype=np.int32))
    in_maps = []
    for i in range(N_CORES):
        yps = np.ascontiguousarray(yp[NB * i : NB * (i + 1)]).reshape(NB, C, NCHUNK, P, F)
        yts = np.ascontiguousarray(yt[NB * i : NB * (i + 1)]).reshape(NB, NCHUNK, P, F)
        in_maps.append({"yp": yps, "yt": yts})
    return in_maps


def epilogue(results):
    """Combine the 8 cores' partial sums into the final dice mean (float32,
    mirroring the reference arithmetic)."""
    tp = np.zeros(7, dtype=np.float64)
    pred_cnt = np.zeros(7, dtype=np.float64)
    gt_cnt = np.zeros(7, dtype=np.float64)
    for r in results:
        tp += np.trace(np.asarray(r["tp_out"], dtype=np.float64), axis1=1, axis2=2)
        pa = np.asarray(r["pa_out"], dtype=np.float64)  # [P, CHUNKS*7]
        ga = np.asarray(r["ga_out"], dtype=np.float64)
        pred_cnt += pa.reshape(P, CHUNKS, 7).sum(axis=(0, 1))
        gt_cnt += ga.reshape(P, CHUNKS, 7).sum(axis=(0, 1))

    tp32 = tp.astype(np.float32)
    fp32_ = (pred_cnt - tp).astype(np.float32)
    fn32 = (gt_cnt - tp).astype(np.float32)
    eps = np.float32(EPS)
    two = np.float32(2.0)
    dice = (two * tp32 + eps) / (two * tp32 + fp32_ + fn32 + eps)
    return np.asarray(np.mean(dice, dtype=np.float32), dtype=np.float32)


def kernel(**inputs):
    from concourse.bass_utils import run_bass_kernel_spmd

    nc = _get_bass()
    in_maps = make_in_maps(inputs["y_true"], inputs["y_pred"])
    res = run_bass_kernel_spmd(nc, in_maps, core_ids=list(range(N_CORES)))
    return epilogue(res.results)


if __name__ == "__main__":
    # smoke test with random data
    rng = np.random.default_rng(0)
    y_true = rng.integers(0, C, size=(16, 512, 512)).astype(np.int32)
    y_pred = rng.standard_normal((16, C, 512, 512)).astype(np.float32)
    out = kernel(y_true=y_true, y_pred=y_pred)
    print("kernel output:", out)



# revision 2
# speedup vs baseline: 1.4808x; 1.4808x over previous
"""Trainium2 Bass kernel for DiceLoss (hard-argmax dice, ignore background, mean).

Problem (hardcoded shapes):
  y_true: [16, 512, 512] int32 in [0, 8)
  y_pred: [16, 8, 512, 512] float32
  out   : scalar float32 = mean over classes 1..7 of
          (2*tp + eps) / (2*tp + fp + fn + eps)
        = (2*tp + eps) / (pred_cnt + gt_cnt + eps)

Strategy (8 NeuronCores, data-parallel over batch; 2 images/core):
  - Streams image planes as [128, 1024] chunks (contiguous HBM DMA).
  - ScalarE: f32->fp16 channel converts + int32->fp16 label convert.
  - DVE (all fp16, 2x/4x perf modes): 7-op pairwise max tree; per class
    pred_c = (ch[c] == m) via tensor_tensor is_equal written into a
    [128, 8, 129] layout whose group-col 0 holds ones; gt_c = (y == c)
    via tensor_scalar is_equal (4x mode, flat [128, 1024]).
  - PE: per (class, chunk, subtile) one matmul
        psum[:, 0:129] += gt_s^T @ [ones | pred_s]
    giving gt colsums in col 0 and tp on the shifted diagonal; plus per
    (class, chunk) 4 ones-stationary colsum matmuls
        psum[0:1, 256:512] += ones^T @ pred(2 groups of 128)
    giving pred counts. Both regions share one PSUM bank per class; only
    the very first matmul into a bank sets start (bank-wide zero).
  - Host: combines the 8 cores' exact f32 count sums into the dice mean.
"""

import numpy as np

EPS = 1e-05

N_CORES = 8
NB = 2          # batch images per core
C = 8           # classes
P = 128         # SBUF partitions
F = 1024        # free-dim elements per chunk
NCHUNK = 2      # chunks per image plane (512*512 = 2*128*1024)
CHUNKS = NB * NCHUNK
NSUB = F // 128  # 8 subtiles per chunk

_CACHED_NC = None


def build_bass():
    from contextlib import ExitStack

    import concourse.bacc as bacc
    import concourse.tile as tile
    from concourse import mybir

    AL = mybir.AluOpType
    ACT = mybir.ActivationFunctionType

    nc = bacc.Bacc(None, target_bir_lowering=False)

    yp = nc.dram_tensor(
        "yp", [NB, C, NCHUNK, P, F], mybir.dt.float32, kind="ExternalInput"
    )
    yt = nc.dram_tensor("yt", [NB, NCHUNK, P, F], mybir.dt.int32, kind="ExternalInput")
    # per class: [128, 129] A-region (col0 = gt colsums, diag = tp)
    a_out = nc.dram_tensor("a_out", [7, P, 129], mybir.dt.float32, kind="ExternalOutput")
    # per class: [256] pred-count partial colsums
    b_out = nc.dram_tensor("b_out", [7, 256], mybir.dt.float32, kind="ExternalOutput")

    with tile.TileContext(nc) as tc, ExitStack() as ctx:
        chpool = ctx.enter_context(tc.tile_pool(name="ch", bufs=2))
        chfpool = ctx.enter_context(tc.tile_pool(name="chf", bufs=2))
        tpool = ctx.enter_context(tc.tile_pool(name="tt", bufs=2))
        mtmp = ctx.enter_context(tc.tile_pool(name="mtmp", bufs=2))
        mpool = ctx.enter_context(tc.tile_pool(name="mx", bufs=2))
        gtpool = ctx.enter_context(tc.tile_pool(name="gt", bufs=2))
        # two fixed pred-tile sets (manual double buffer, ones cols set once)
        predpA = ctx.enter_context(tc.tile_pool(name="pdA", bufs=1))
        predpB = ctx.enter_context(tc.tile_pool(name="pdB", bufs=1))
        onesp = ctx.enter_context(tc.tile_pool(name="on", bufs=1))
        psump = ctx.enter_context(tc.tile_pool(name="psum", bufs=1, space="PSUM"))

        banks = [
            psump.tile([P, 512], mybir.dt.float32, name=f"bk{c}", tag=f"bk{c}")
            for c in range(7)
        ]

        ones1 = onesp.tile([P, 1], mybir.dt.float16, name="ones1")
        nc.vector.memset(ones1, 1.0)

        predA = [
            predpA.tile([P, NSUB, 129], mybir.dt.float16, name=f"pA{c}", tag=f"pA{c}")
            for c in range(1, C)
        ]
        predB = [
            predpB.tile([P, NSUB, 129], mybir.dt.float16, name=f"pB{c}", tag=f"pB{c}")
            for c in range(1, C)
        ]
        for t in predA + predB:
            nc.vector.memset(t[:, :, 0:1], 1.0)

        chunk_idx = 0
        for n in range(NB):
            for j in range(NCHUNK):
                first = chunk_idx == 0
                last = chunk_idx == CHUNKS - 1
                preds = predA if chunk_idx % 2 == 0 else predB

                ch = []
                for c in range(C):
                    tl = chpool.tile([P, F], mybir.dt.float32, name=f"ch{c}", tag=f"ch{c}")
                    nc.sync.dma_start(out=tl, in_=yp[n, c, j])
                    ch.append(tl)
                tt_ = tpool.tile([P, F], mybir.dt.int32, name="t", tag="t")
                nc.sync.dma_start(out=tt_, in_=yt[n, j])

                # ---- ScalarE: converts ----
                chf = []
                for c in range(C):
                    tf = chfpool.tile([P, F], mybir.dt.float16, name=f"cf{c}", tag=f"cf{c}")
                    nc.scalar.activation(out=tf, in_=ch[c], func=ACT.Copy)
                    chf.append(tf)
                yf = tpool.tile([P, F], mybir.dt.float16, name="yf", tag="yf")
                nc.scalar.activation(out=yf, in_=tt_, func=ACT.Copy)

                # ---- DVE: max tree (fp16 tensor_tensor, 2x) ----
                m01 = mtmp.tile([P, F], mybir.dt.float16, name="m01", tag="m01")
                nc.vector.tensor_max(m01, chf[0], chf[1])
                m23 = mtmp.tile([P, F], mybir.dt.float16, name="m23", tag="m23")
                nc.vector.tensor_max(m23, chf[2], chf[3])
                m45 = mtmp.tile([P, F], mybir.dt.float16, name="m45", tag="m45")
                nc.vector.tensor_max(m45, chf[4], chf[5])
                m67 = mtmp.tile([P, F], mybir.dt.float16, name="m67", tag="m67")
                nc.vector.tensor_max(m67, chf[6], chf[7])
                m0123 = mtmp.tile([P, F], mybir.dt.float16, name="m0123", tag="m01")
                nc.vector.tensor_max(m0123, m01, m23)
                m4567 = mtmp.tile([P, F], mybir.dt.float16, name="m4567", tag="m45")
                nc.vector.tensor_max(m4567, m45, m67)
                m = mpool.tile([P, F], mybir.dt.float16, name="m", tag="m")
                nc.vector.tensor_max(m, m0123, m4567)

                # ---- per class: pred mask (tt is_equal, 2x), gt mask (ts, 4x), PE ----
                for c in range(1, C):
                    px = preds[c - 1]
                    nc.vector.tensor_tensor(
                        out=px[:, :, 1:129], in0=chf[c], in1=m, op=AL.is_equal
                    )
                    gt = gtpool.tile([P, F], mybir.dt.float16, name=f"gt{c}", tag=f"gt{c}")
                    nc.vector.tensor_scalar(
                        out=gt, in0=yf, scalar1=float(c), scalar2=0.0,
                        op0=AL.is_equal, op1=AL.add,
                    )
                    bank = banks[c - 1]
                    for s in range(NSUB):
                        nc.tensor.matmul(
                            bank[:, 0:129],
                            lhsT=gt[:, s * 128:(s + 1) * 128],
                            rhs=px[:, s, 0:129],
                            start=(first and s == 0),
                            stop=(last and s == NSUB - 1),
                            skip_group_check=True,
                        )
                    for hf in range(4):
                        nc.tensor.matmul(
                            bank[0:1, 256:512],
                            lhsT=ones1[:, 0:1],
                            rhs=px[:, 2 * hf:2 * hf + 2, 1:129],
                            start=False,
                            stop=(last and hf == 3),
                            skip_group_check=True,
                        )
                chunk_idx += 1

        # ---- readback: PSUM -> SBUF (ScalarE) -> DRAM ----
        outp = ctx.enter_context(tc.tile_pool(name="out", bufs=1))
        for c in range(7):
            oa = outp.tile([P, 129], mybir.dt.float32, name=f"oa{c}", tag=f"oa{c}")
            nc.scalar.copy(out=oa, in_=banks[c][:, 0:129])
            nc.sync.dma_start(out=a_out[c], in_=oa)
            ob = outp.tile([1, 256], mybir.dt.float32, name=f"ob{c}", tag=f"ob{c}")
            nc.scalar.copy(out=ob, in_=banks[c][0:1, 256:512])
            nc.sync.dma_start(out=b_out[c], in_=ob)

    nc.finalize()
    return nc


def _get_bass():
    global _CACHED_NC
    if _CACHED_NC is None:
        _CACHED_NC = build_bass()
    return _CACHED_NC


def make_in_maps(y_true, y_pred):
    yp = np.ascontiguousarray(np.asarray(y_pred, dtype=np.float32))
    yt = np.ascontiguousarray(np.asarray(y_true, dtype=np.int32))
    in_maps = []
    for i in range(N_CORES):
        yps = np.ascontiguousarray(yp[NB * i: NB * (i + 1)]).reshape(NB, C, NCHUNK, P, F)
        yts = np.ascontiguousarray(yt[NB * i: NB * (i + 1)]).reshape(NB, NCHUNK, P, F)
        in_maps.append({"yp": yps, "yt": yts})
    return in_maps


def epilogue(results):
    """Combine the 8 cores' exact f32 partial sums into the dice mean."""
    tp = np.zeros(7, dtype=np.float64)
    gt_cnt = np.zeros(7, dtype=np.float64)
    pred_cnt = np.zeros(7, dtype=np.float64)
    idx = np.arange(128)
    for r in results:
        a = np.asarray(r["a_out"], dtype=np.float64)   # [7, 128, 129]
        b = np.asarray(r["b_out"], dtype=np.float64)   # [7, 256]
        gt_cnt += a[:, :, 0].sum(axis=1)
        tp += a[:, idx, 1 + idx].sum(axis=1)
        pred_cnt += b.sum(axis=1)

    tp32 = tp.astype(np.float32)
    denom = (pred_cnt + gt_cnt).astype(np.float32)
    eps = np.float32(EPS)
    two = np.float32(2.0)
    dice = (two * tp32 + eps) / (denom + eps)
    return np.asarray(np.mean(dice, dtype=np.float32), dtype=np.float32)


def kernel(**inputs):
    from concourse.bass_utils import run_bass_kernel_spmd

    nc = _get_bass()
    in_maps = make_in_maps(inputs["y_true"], inputs["y_pred"])
    res = run_bass_kernel_spmd(nc, in_maps, core_ids=list(range(N_CORES)))
    return epilogue(res.results)


if __name__ == "__main__":
    rng = np.random.default_rng(0)
    y_true = rng.integers(0, C, size=(16, 512, 512)).astype(np.int32)
    y_pred = rng.standard_normal((16, C, 512, 512)).astype(np.float32)
    out = kernel(y_true=y_true, y_pred=y_pred)
    print("kernel output:", out)

    # numpy oracle
    pred_cls = np.argmax(y_pred, axis=1)
    tp = np.zeros(7); fp = np.zeros(7); fn = np.zeros(7)
    for c in range(1, 8):
        pm = pred_cls == c
        gm = y_true == c
        tp[c-1] = np.sum(pm & gm)
        fp[c-1] = np.sum(pm & ~gm)
        fn[c-1] = np.sum(~pm & gm)
    dice = (2*tp + EPS) / (2*tp + fp + fn + EPS)
    print("numpy oracle:", dice.mean())


# revision 3
# speedup vs baseline: 1.5776x; 1.0653x over previous
"""Trainium2 Bass kernel for DiceLoss (hard-argmax dice, ignore background, mean).

Problem (hardcoded shapes):
  y_true: [16, 512, 512] int32 in [0, 8)
  y_pred: [16, 8, 512, 512] float32
  out   : scalar float32 = mean over classes 1..7 of
          (2*tp + eps) / (2*tp + fp + fn + eps)
        = (2*tp + eps) / (pred_cnt + gt_cnt + eps)

Strategy (8 NeuronCores, data-parallel over batch; 2 images/core):
  - Streams image planes as [128, 1024] chunks (contiguous HBM DMA).
  - ScalarE: f32->fp16 channel converts + int32->fp16 label convert.
  - DVE (all fp16, 2x/4x perf modes): 7-op pairwise max tree; per class
    pred_c = (ch[c] == m) via tensor_tensor is_equal written into a
    [128, 8, 129] layout whose group-col 0 holds ones; gt_c = (y == c)
    via tensor_scalar is_equal (4x mode, flat [128, 1024]).
  - PE: per (class, chunk, subtile) one matmul
        psum[:, 0:129] += gt_s^T @ [ones | pred_s]
    giving gt colsums in col 0 and tp on the shifted diagonal; plus per
    (class, chunk) 4 ones-stationary colsum matmuls
        psum[0:1, 256:512] += ones^T @ pred(2 groups of 128)
    giving pred counts. Both regions share one PSUM bank per class; only
    the very first matmul into a bank sets start (bank-wide zero).
  - Host: combines the 8 cores' exact f32 count sums into the dice mean.
"""

import numpy as np

EPS = 1e-05

N_CORES = 8
NB = 2          # batch images per core
C = 8           # classes
P = 128         # SBUF partitions
F = 1024        # free-dim elements per chunk
NCHUNK = 2      # chunks per image plane (512*512 = 2*128*1024)
CHUNKS = NB * NCHUNK
NSUB = F // 128  # 8 subtiles per chunk

_CACHED_NC = None


def build_bass():
    from contextlib import ExitStack

    import concourse.bacc as bacc
    import concourse.tile as tile
    from concourse import mybir

    AL = mybir.AluOpType
    ACT = mybir.ActivationFunctionType

    nc = bacc.Bacc(None, target_bir_lowering=False)

    yp = nc.dram_tensor(
        "yp", [NB, C, NCHUNK, P, F], mybir.dt.float32, kind="ExternalInput"
    )
    yt = nc.dram_tensor("yt", [NB, NCHUNK, P, F], mybir.dt.int32, kind="ExternalInput")
    # per class: [128, 129] A-region (col0 = gt colsums, diag = tp)
    a_out = nc.dram_tensor("a_out", [7, P, 129], mybir.dt.float32, kind="ExternalOutput")
    # pred-count partial colsums: row c-1 = class c
    b_out = nc.dram_tensor("b_out", [7, 512], mybir.dt.float32, kind="ExternalOutput")

    with tile.TileContext(nc) as tc, ExitStack() as ctx:
        chpool = ctx.enter_context(tc.tile_pool(name="ch", bufs=2))
        chfpool = ctx.enter_context(tc.tile_pool(name="chf", bufs=2))
        tpool = ctx.enter_context(tc.tile_pool(name="tt", bufs=2))
        mtmp = ctx.enter_context(tc.tile_pool(name="mtmp", bufs=2))
        mpool = ctx.enter_context(tc.tile_pool(name="mx", bufs=2))
        gtpool = ctx.enter_context(tc.tile_pool(name="gt", bufs=2))
        # two fixed pred-tile sets (manual double buffer, ones cols set once)
        predpA = ctx.enter_context(tc.tile_pool(name="pdA", bufs=1))
        predpB = ctx.enter_context(tc.tile_pool(name="pdB", bufs=1))
        onesp = ctx.enter_context(tc.tile_pool(name="on", bufs=1))
        psump = ctx.enter_context(tc.tile_pool(name="psum", bufs=1, space="PSUM"))

        banks = [
            psump.tile([P, 512], mybir.dt.float32, name=f"bk{c}", tag=f"bk{c}")
            for c in range(7)
        ]
        bbank = psump.tile([P, 512], mybir.dt.float32, name="bb", tag="bb")

        # one-hot stationary columns: ohs[c-1][:, c-1] = 1 for class c
        ohs = []
        for c in range(7):
            t = onesp.tile([P, 7], mybir.dt.float16, name=f"oh{c}")
            nc.vector.memset(t, 0.0)
            nc.vector.memset(t[:, c:c + 1], 1.0)
            ohs.append(t)

        predA = [
            predpA.tile([P, NSUB, 129], mybir.dt.float16, name=f"pA{c}", tag=f"pA{c}")
            for c in range(1, C)
        ]
        predB = [
            predpB.tile([P, NSUB, 129], mybir.dt.float16, name=f"pB{c}", tag=f"pB{c}")
            for c in range(1, C)
        ]
        for t in predA + predB:
            nc.vector.memset(t[:, :, 0:1], 1.0)

        chunk_idx = 0
        for n in range(NB):
            for j in range(NCHUNK):
                first = chunk_idx == 0
                last = chunk_idx == CHUNKS - 1
                preds = predA if chunk_idx % 2 == 0 else predB

                ch = []
                for c in range(C):
                    tl = chpool.tile([P, F], mybir.dt.float32, name=f"ch{c}", tag=f"ch{c}")
                    nc.sync.dma_start(out=tl, in_=yp[n, c, j])
                    ch.append(tl)
                tt_ = tpool.tile([P, F], mybir.dt.int32, name="t", tag="t")
                nc.sync.dma_start(out=tt_, in_=yt[n, j])

                # ---- ScalarE: converts ----
                chf = []
                for c in range(C):
                    tf = chfpool.tile([P, F], mybir.dt.float16, name=f"cf{c}", tag=f"cf{c}")
                    nc.scalar.activation(out=tf, in_=ch[c], func=ACT.Copy)
                    chf.append(tf)
                yf = tpool.tile([P, F], mybir.dt.float16, name="yf", tag="yf")
                nc.scalar.activation(out=yf, in_=tt_, func=ACT.Copy)

                # ---- DVE: max tree (fp16 tensor_tensor, 2x) ----
                m01 = mtmp.tile([P, F], mybir.dt.float16, name="m01", tag="m01")
                nc.vector.tensor_max(m01, chf[0], chf[1])
                m23 = mtmp.tile([P, F], mybir.dt.float16, name="m23", tag="m23")
                nc.vector.tensor_max(m23, chf[2], chf[3])
                m45 = mtmp.tile([P, F], mybir.dt.float16, name="m45", tag="m45")
                nc.vector.tensor_max(m45, chf[4], chf[5])
                m67 = mtmp.tile([P, F], mybir.dt.float16, name="m67", tag="m67")
                nc.vector.tensor_max(m67, chf[6], chf[7])
                m0123 = mtmp.tile([P, F], mybir.dt.float16, name="m0123", tag="m01")
                nc.vector.tensor_max(m0123, m01, m23)
                m4567 = mtmp.tile([P, F], mybir.dt.float16, name="m4567", tag="m45")
                nc.vector.tensor_max(m4567, m45, m67)
                m = mpool.tile([P, F], mybir.dt.float16, name="m", tag="m")
                nc.vector.tensor_max(m, m0123, m4567)

                # ---- per class: pred mask (tt is_equal, 2x), gt mask (ts, 4x) ----
                gts = []
                for c in range(1, C):
                    px = preds[c - 1]
                    nc.vector.tensor_tensor(
                        out=px[:, :, 1:129], in0=chf[c], in1=m, op=AL.is_equal
                    )
                    gt = gtpool.tile([P, F], mybir.dt.float16, name=f"gt{c}", tag=f"gt{c}")
                    nc.vector.tensor_scalar(
                        out=gt, in0=yf, scalar1=float(c), scalar2=0.0,
                        op0=AL.is_equal, op1=AL.add,
                    )
                    gts.append(gt)

                # ---- PE: diag MMs subtile-outer (consecutive MMs hit
                # different banks and pipeline), then shared-bank colsums ----
                for s in range(NSUB):
                    for c in range(1, C):
                        nc.tensor.matmul(
                            banks[c - 1][:, 0:129],
                            lhsT=gts[c - 1][:, s * 128:(s + 1) * 128],
                            rhs=preds[c - 1][:, s, 0:129],
                            start=(first and s == 0),
                            stop=(last and s == NSUB - 1),
                            skip_group_check=True,
                        )
                for c in range(1, C):
                    for hf in range(2):
                        nc.tensor.matmul(
                            bbank[0:7, 0:512],
                            lhsT=ohs[c - 1][:, 0:7],
                            rhs=preds[c - 1][:, 4 * hf:4 * hf + 4, 1:129],
                            start=(first and c == 1 and hf == 0),
                            stop=(last and c == C - 1 and hf == 1),
                            skip_group_check=True,
                        )
                chunk_idx += 1

        # ---- readback: PSUM -> SBUF (ScalarE) -> DRAM ----
        outp = ctx.enter_context(tc.tile_pool(name="out", bufs=1))
        for c in range(7):
            oa = outp.tile([P, 129], mybir.dt.float32, name=f"oa{c}", tag=f"oa{c}")
            nc.scalar.copy(out=oa, in_=banks[c][:, 0:129])
            nc.sync.dma_start(out=a_out[c], in_=oa)
        ob = outp.tile([7, 512], mybir.dt.float32, name="ob", tag="ob")
        nc.scalar.copy(out=ob, in_=bbank[0:7, 0:512])
        nc.sync.dma_start(out=b_out[:, :], in_=ob)

    nc.finalize()
    return nc


def _get_bass():
    global _CACHED_NC
    if _CACHED_NC is None:
        _CACHED_NC = build_bass()
    return _CACHED_NC


def make_in_maps(y_true, y_pred):
    yp = np.ascontiguousarray(np.asarray(y_pred, dtype=np.float32))
    yt = np.ascontiguousarray(np.asarray(y_true, dtype=np.int32))
    in_maps = []
    for i in range(N_CORES):
        yps = np.ascontiguousarray(yp[NB * i: NB * (i + 1)]).reshape(NB, C, NCHUNK, P, F)
        yts = np.ascontiguousarray(yt[NB * i: NB * (i + 1)]).reshape(NB, NCHUNK, P, F)
        in_maps.append({"yp": yps, "yt": yts})
    return in_maps


def epilogue(results):
    """Combine the 8 cores' exact f32 partial sums into the dice mean."""
    tp = np.zeros(7, dtype=np.float64)
    gt_cnt = np.zeros(7, dtype=np.float64)
    pred_cnt = np.zeros(7, dtype=np.float64)
    idx = np.arange(128)
    for r in results:
        a = np.asarray(r["a_out"], dtype=np.float64)   # [7, 128, 129]
        b = np.asarray(r["b_out"], dtype=np.float64)   # [7, 512]
        gt_cnt += a[:, :, 0].sum(axis=1)
        tp += a[:, idx, 1 + idx].sum(axis=1)
        pred_cnt += b.sum(axis=1)

    tp32 = tp.astype(np.float32)
    denom = (pred_cnt + gt_cnt).astype(np.float32)
    eps = np.float32(EPS)
    two = np.float32(2.0)
    dice = (two * tp32 + eps) / (denom + eps)
    return np.asarray(np.mean(dice, dtype=np.float32), dtype=np.float32)


def kernel(**inputs):
    from concourse.bass_utils import run_bass_kernel_spmd

    nc = _get_bass()
    in_maps = make_in_maps(inputs["y_true"], inputs["y_pred"])
    res = run_bass_kernel_spmd(nc, in_maps, core_ids=list(range(N_CORES)))
    return epilogue(res.results)


if __name__ == "__main__":
    rng = np.random.default_rng(0)
    y_true = rng.integers(0, C, size=(16, 512, 512)).astype(np.int32)
    y_pred = rng.standard_normal((16, C, 512, 512)).astype(np.float32)
    out = kernel(y_true=y_true, y_pred=y_pred)
    print("kernel output:", out)

    # numpy oracle
    pred_cls = np.argmax(y_pred, axis=1)
    tp = np.zeros(7); fp = np.zeros(7); fn = np.zeros(7)
    for c in range(1, 8):
        pm = pred_cls == c
        gm = y_true == c
        tp[c-1] = np.sum(pm & gm)
        fp[c-1] = np.sum(pm & ~gm)
        fn[c-1] = np.sum(~pm & gm)
    dice = (2*tp + EPS) / (2*tp + fp + fn + EPS)
    print("numpy oracle:", dice.mean())
